# revision 14
# baseline (speedup 1.0000x reference)
"""MoE (top-2 of 8 experts, D=768, FF=3072) on 8 Trainium2 NeuronCores.

Strategy: FF-slice parallelism. The router runs on host; every core holds a
384-wide slice of D_FF for ALL 8 experts (same 9.4MB of fp16 weights per core
as expert-parallel) and runs the FFN for ALL routed token-expert pairs over
its slice, one expert "phase" at a time. Each core therefore does exactly
sum(C_e)*36 matmul-cycles -- perfect load balance with zero padding (vs
8*max(C_e)*36 for expert-parallel). The host sums the 8 partial outputs
(fp16) and applies the softmax-weighted combine + b2.

Device layout puts tokens on the matmul free axis, so both matmuls contract
over the partition axis with zero on-device transposes:
    HT[f,t] = relu(sum_d W1[d,f] * XT[d,t] + b1[f])   lhsT=W1, rhs=XT
    YT[d,t] =      sum_{f in slice} W2[f,d] * HT[f,t] lhsT=W2, rhs=HT
The chunk loop is software-pipelined (MM1 of chunk i+1 is emitted before
MM2 of chunk i) so the relu latency never stalls the PE. MM2's PSUM
accumulation is only 3 deep here, so the drain load is 8x expert-parallel's;
relu runs on ACT and the MM2 drains split 4:2 between DVE and ACT (Pool
cannot read PSUM). Tokens stream on the sync DMA queue, weights on the DVE
queue (parallel transfer in the prologue), outputs issue from Pool. The
last items' outputs go out per-2ko so the final transfers overlap the tail.
"""

import numpy as np

import concourse.tile as tile
from concourse import bacc, mybir
from concourse import bass_utils

D_MODEL = 768
N_EXPERTS = 8
TOP_K = 2
D_FF = 3072
P = 128
KO = D_MODEL // P     # 6   contraction chunks for MM1 / output tiles for MM2
FS = 3                # f-tiles per core slice (384 of 3072 FF columns)
FSP = FS * P          # 384
N0 = 256              # first-phase chunk0 (small so its tokens land early)
WARMUP_MMS = 38       # small 128-row dummy matmuls bridge the DMA prologue
TAIL_SPLIT = 3        # last N work items DMA their output per-2ko

_program_cache: dict[tuple, object] = {}


def _phase_chunks(C, first):
    """Chunks (t0, nt) covering C tokens, each <=512. The first phase opens
    with a small N0 chunk so the critical DMAs land early."""
    chunks = []
    t = 0
    if first and C > N0:
        chunks.append((0, N0))
        t = N0
    while t < C:
        n = min(512, C - t)
        chunks.append((t, n))
        t += n
    return chunks


def _build_program(counts):
    """Bass program: all 8 expert phases over this core's FF slice (SPMD x8).

    counts: per-phase token counts (already padded to x4)."""
    key = tuple(counts)
    if key in _program_cache:
        return _program_cache[key]

    fp16 = mybir.dt.float16
    fp32 = mybir.dt.float32
    nc = bacc.Bacc("TRN2", target_bir_lowering=False, debug=False,
                   enable_asserts=True, num_devices=N_EXPERTS)

    cmax = max(counts)
    c0 = counts[0]
    # wb_e layout: w1_e (ko-major [KO, FSP]) then w2_e ([FS, D])
    off_w2 = KO * FSP
    lw = off_w2 + FS * D_MODEL

    xta_d = nc.dram_tensor("xta", [P, KO, N0], fp16,
                           kind="ExternalInput").ap()
    xtb_d = nc.dram_tensor("xtb", [P, KO, c0 - N0], fp16,
                           kind="ExternalInput").ap()
    b1_d = nc.dram_tensor("b1c", [P, N_EXPERTS * FS], fp32,
                          kind="ExternalInput").ap()
    w1a_d = nc.dram_tensor("w1a", [P, KO, P], fp16,
                           kind="ExternalInput").ap()
    w1b_d = nc.dram_tensor("w1b", [P, KO, (FS - 1) * P], fp16,
                           kind="ExternalInput").ap()
    w2a_d = nc.dram_tensor("w2a", [P, FS, D_MODEL], fp16,
                           kind="ExternalInput").ap()
    wb_d = [None] + [
        nc.dram_tensor(f"wb{e}", [P, lw], fp16, kind="ExternalInput").ap()
        for e in range(1, N_EXPERTS)]
    xt_d = [None] + [
        nc.dram_tensor(f"xt{e}", [P, KO, counts[e]], fp16,
                       kind="ExternalInput").ap()
        for e in range(1, N_EXPERTS)]
    yt_d = [nc.dram_tensor(f"yt{e}", [P, KO, counts[e]], fp16,
                           kind="ExternalOutput").ap()
            for e in range(N_EXPERTS)]

    with tile.TileContext(nc) as tc:
        with (
            tc.tile_pool(name="wpool", bufs=1) as wpool,
            tc.tile_pool(name="xtpool", bufs=4) as xtpool,
            tc.tile_pool(name="hpool", bufs=2) as hpool,
            tc.tile_pool(name="ypool", bufs=3) as ypool,
            tc.tile_pool(name="pspool", bufs=7, space="PSUM") as pspool,
        ):
            xta_sb = wpool.tile([P, KO, N0], fp16)
            xtb_sb = wpool.tile([P, KO, c0 - N0], fp16)
            w1a_sb = wpool.tile([P, KO, P], fp16)
            w1b_sb = wpool.tile([P, KO, (FS - 1) * P], fp16)
            w2a_sb = wpool.tile([P, FS, D_MODEL], fp16)
            wb_sb = [None] + [wpool.tile([P, lw], fp16, name=f"wb_sb{e}")
                              for e in range(1, N_EXPERTS)]
            b1_sb = wpool.tile([P, N_EXPERTS * FS], fp32)

            def w1_slice(e, ft, ko):
                """AP of w1 f-tile `ft` (local), contraction chunk ko."""
                if e == 0:
                    if ft == 0:
                        return w1a_sb[:, ko, :]
                    return w1b_sb[:, ko, (ft - 1) * P:ft * P]
                o = ko * FSP + ft * P
                return wb_sb[e][:, o:o + P]

            def w2_slice(e, ft, ko):
                if e == 0:
                    return w2a_sb[:, ft, ko * P:(ko + 1) * P]
                o = off_w2 + ft * D_MODEL + ko * P
                return wb_sb[e][:, o:o + P]

            # PE warmup: small dummy matmuls bridge the DMA prologue and keep
            # the HAM clock ramping; small so the first real matmul slots in
            # quickly once the critical DMAs land.
            warm = wpool.tile([P, P], fp16)
            hdummy = wpool.tile([P, 8], fp16)
            nc.gpsimd.memset(warm[:], 0.0)
            # dummy activation: forces the 1.3us ACT_TABLE_LOAD into the
            # prologue instead of blocking the first real relu
            nc.scalar.activation(hdummy[:], warm[:, :8],
                                 mybir.ActivationFunctionType.Relu)
            ps_w = pspool.tile([P, 512], fp32, name="ps_w", bufs=1)
            for _ in range(WARMUP_MMS):
                nc.tensor.matmul(ps_w[:, :P], lhsT=warm[:], rhs=warm[:],
                                 start=True, stop=True)

            # Input DMAs: tokens on the sync queue, weights on the scalar
            # (ACT) queue, each in stream (need) order; the two queues
            # transfer in parallel so the first phase's tokens AND weights
            # land early. Only phase-0's small weight pieces are issued
            # up-front; w2a and the wb bundles are issued from inside the
            # compute stream (one phase ahead) so they don't delay the
            # first relus on the ACT queue. The xt pool's bufs=4 rotation
            # paces the token stream.
            nc.scalar.dma_start(xta_sb[:], xta_d[:])
            nc.scalar.dma_start(xtb_sb[:], xtb_d[:])
            nc.sync.dma_start(b1_sb[:], b1_d[:])
            nc.sync.dma_start(w1a_sb[:], w1a_d[:])
            nc.sync.dma_start(w1b_sb[:], w1b_d[:])
            nc.sync.dma_start(w2a_sb[:], w2a_d[:])
            # xt tiles are allocated here (pool rotation order) but their
            # DMAs are issued from inside the compute stream, a phase ahead,
            # so they don't steal prologue bandwidth from the critical path
            xt_sb = [None] * N_EXPERTS
            for e in range(1, N_EXPERTS):
                xt_sb[e] = xtpool.tile([P, KO, cmax], fp16, name="xt")

            def xt_rhs(e, ko, t0, nt):
                if e == 0:
                    if t0 < N0:
                        return xta_sb[:, ko, t0:t0 + nt]
                    return xtb_sb[:, ko, t0 - N0:t0 - N0 + nt]
                return xt_sb[e][:, ko, t0:t0 + nt]

            # work items: (phase, t0, nt), software-pipelined one deep
            items = []
            first_item = {}
            fidx = {}
            for e in range(N_EXPERTS):
                for (t0, nt) in _phase_chunks(counts[e], first=(e == 0)):
                    if t0 == 0:
                        first_item[len(items)] = e
                        fidx[e] = len(items)
                    items.append((e, t0, nt))
            # anchor item at which each phase's token DMA is issued (a phase
            # ahead of need; phase 1's waits one extra chunk so it doesn't
            # steal prologue bandwidth from the critical w1/xt transfers)
            issue_xt = {}
            for p in range(1, N_EXPERTS):
                anchor = fidx[p - 1] + 1 if p == 1 else fidx[p - 1]
                issue_xt.setdefault(anchor, []).append(p)

            hts = {}

            def emit_mm1(i):
                e, t0, nt = items[i]
                ht = hpool.tile([P, FS, 512], fp16, name="ht")
                hts[i] = ht
                for ft in range(FS):
                    ps = pspool.tile([P, 512], fp32, name="ps")
                    for ko in range(KO):
                        nc.tensor.matmul(
                            ps[:, :nt],
                            lhsT=w1_slice(e, ft, ko),
                            rhs=xt_rhs(e, ko, t0, nt),
                            start=(ko == 0), stop=(ko == KO - 1),
                        )
                    nc.scalar.activation(
                        ht[:, ft, :nt], ps[:, :nt],
                        mybir.ActivationFunctionType.Relu,
                        bias=b1_sb[:, e * FS + ft:e * FS + ft + 1],
                    )
                # issue the next phase's weight bundle from the ACT queue
                # and its tokens from the sync queue (behind this item's
                # relus, one phase ahead of need)
                if i in first_item:
                    p = first_item[i]
                    if p + 1 < N_EXPERTS:
                        nc.scalar.dma_start(wb_sb[p + 1][:], wb_d[p + 1][:])
                for p in issue_xt.get(i, ()):
                    nc.sync.dma_start(xt_sb[p][:, :, :counts[p]], xt_d[p][:])

            def emit_mm2(i):
                e, t0, nt = items[i]
                ht = hts.pop(i)
                tail = i >= len(items) - TAIL_SPLIT
                yt = ypool.tile([P, KO, 512], fp16, name="yt")
                for ko in range(KO):
                    ps = pspool.tile([P, 512], fp32, name="ps")
                    for ft in range(FS):
                        nc.tensor.matmul(
                            ps[:, :nt],
                            lhsT=w2_slice(e, ft, ko),
                            rhs=ht[:, ft, :nt],
                            start=(ft == 0), stop=(ft == FS - 1),
                        )
                    # MM2's drain load is heavy (3-deep accumulation) and
                    # Pool can't read PSUM: split it 4:2 across DVE and ACT
                    if ko % 3:
                        nc.vector.tensor_copy(yt[:, ko, :nt], ps[:, :nt])
                    else:
                        nc.scalar.activation(
                            yt[:, ko, :nt], ps[:, :nt],
                            mybir.ActivationFunctionType.Copy)
                    if tail and ko % 2 == 1:
                        # tail items stream their output out per-2ko, spread
                        # over three otherwise-idle queues, so the final
                        # transfers and their issue overlap remaining compute
                        eng = (nc.sync, nc.gpsimd, nc.scalar)[ko // 2]
                        eng.dma_start(
                            yt_d[e][:, ko - 1:ko + 1, t0:t0 + nt],
                            yt[:, ko - 1:ko + 1, :nt])
                if not tail:
                    nc.gpsimd.dma_start(yt_d[e][:, :, t0:t0 + nt],
                                        yt[:, :, :nt])

            emit_mm1(0)
            for i in range(len(items) - 1):
                emit_mm1(i + 1)
                emit_mm2(i)
            emit_mm2(len(items) - 1)

    nc.compile()
    _program_cache[key] = nc
    return nc


def _route(xf, Wr):
    """Host router: top-2 expert ids + softmax weights (matches lax.top_k)."""
    T = xf.shape[0]
    logits = xf @ Wr
    i1 = np.argmax(logits, axis=1)
    l1 = logits[np.arange(T), i1]
    masked = logits.copy()
    masked[np.arange(T), i1] = -np.inf
    i2 = np.argmax(masked, axis=1)
    l2 = logits[np.arange(T), i2]
    e2 = np.exp((l2 - l1).astype(np.float32))
    wt1 = 1.0 / (1.0 + e2)
    wt2 = e2 / (1.0 + e2)
    return i1, i2, wt1, wt2


def _forward(inputs, trace=False, trace_kwargs=None):
    x = np.ascontiguousarray(np.asarray(inputs["x"], dtype=np.float32))
    Wr = np.asarray(inputs["Wr"], dtype=np.float32)
    W1 = np.asarray(inputs["W1"], dtype=np.float32)
    b1 = np.asarray(inputs["b1"], dtype=np.float32)
    W2 = np.asarray(inputs["W2"], dtype=np.float32)
    b2 = np.asarray(inputs["b2"], dtype=np.float32)

    B, S, D = x.shape
    T = B * S
    xf = x.reshape(T, D)

    i1, i2, wt1, wt2 = _route(xf, Wr)
    idx = [np.nonzero((i1 == e) | (i2 == e))[0] for e in range(N_EXPERTS)]
    gw = [np.where(i1[ix] == e, wt1[ix], wt2[ix]).astype(np.float32)
          for e, ix in enumerate(idx)]

    # phase order: the last phase ends the kernel, so give it the smallest
    # trailing chunk (fast drain tail)
    counts = [max(-(-len(ix) // 4) * 4, 4) for ix in idx]

    def rem(c):
        r = c % 512
        return r if r else 512
    order = list(range(N_EXPERTS))
    last = min(order, key=lambda e: rem(counts[e]))
    order.remove(last)
    order.append(last)

    pcounts = [counts[e] for e in order]
    nc = _build_program(tuple(pcounts))

    # per-phase token tensors (identical for every core)
    xts = []
    for p, e in enumerate(order):
        ix = idx[e]
        C = pcounts[p]
        xe = np.zeros((C, D), dtype=np.float16)
        xe[:len(ix)] = xf[ix]
        # XT[d,t] -> [p, ko, t] with d = ko*P + p
        xts.append(np.ascontiguousarray(
            xe.T.reshape(KO, P, C).transpose(1, 0, 2)))

    in_maps = []
    for c in range(N_EXPERTS):
        fsl = slice(c * FSP, (c + 1) * FSP)
        m = {}
        w1s, w2s, b1s = [], [], []
        for p, e in enumerate(order):
            w1s.append(np.ascontiguousarray(
                W1[e].astype(np.float16).reshape(KO, P, D_FF)[:, :, fsl]
                .transpose(1, 0, 2)))       # [P, KO, FSP]
            w2s.append(np.ascontiguousarray(
                W2[e].astype(np.float16)
                .reshape(D_FF // P, P, D_MODEL)[c * FS:(c + 1) * FS]
                .transpose(1, 0, 2)))       # [P, FS, D]
            b1s.append(b1[e][fsl].reshape(FS, P).T)
        m["xta"] = np.ascontiguousarray(xts[0][:, :, :N0])
        m["xtb"] = np.ascontiguousarray(xts[0][:, :, N0:])
        m["b1c"] = np.ascontiguousarray(np.concatenate(b1s, axis=1))
        m["w1a"] = np.ascontiguousarray(w1s[0][:, :, :P])
        m["w1b"] = np.ascontiguousarray(w1s[0][:, :, P:])
        m["w2a"] = w2s[0]
        for p in range(1, N_EXPERTS):
            m[f"wb{p}"] = np.ascontiguousarray(np.concatenate(
                [w1s[p].reshape(P, -1), w2s[p].reshape(P, -1)], axis=1))
            m[f"xt{p}"] = xts[p]
        in_maps.append(m)

    try:
        res = bass_utils.run_bass_kernel_spmd(
            nc, in_maps, core_ids=list(range(N_EXPERTS)), trace=trace,
            **(trace_kwargs or {}),
        )
    except Exception:
        # transient device errors (NRT_EXEC_UNIT_UNRECOVERABLE) have been
        # observed once under rapid successive loads; one retry clears them
        res = bass_utils.run_bass_kernel_spmd(
            nc, in_maps, core_ids=list(range(N_EXPERTS)), trace=trace,
            **(trace_kwargs or {}),
        )

    out = np.zeros((T, D), dtype=np.float32)
    for p, e in enumerate(order):
        ix = idx[e]
        if len(ix) == 0:
            continue
        # sum the 8 cores' fp16 partials: yt [p, ko, t] -> Y [t, d]
        yt = res.results[0][f"yt{p}"].astype(np.float32)
        for c in range(1, N_EXPERTS):
            yt += res.results[c][f"yt{p}"].astype(np.float32)
        ye = yt.transpose(2, 1, 0).reshape(pcounts[p], D)[:len(ix)]
        out[ix] += gw[e][:, None] * (ye + b2[e][None, :])
    return out.reshape(B, S, D), res


def kernel(**inputs) -> np.ndarray:
    out, _ = _forward(inputs)
    return out


# revision 19
# speedup vs baseline: 1.0283x; 1.0283x over previous
"""MoE (top-2 of 8 experts, D=768, FF=3072) on 8 Trainium2 NeuronCores.

Strategy: FF-slice parallelism. The router runs on host; every core holds a
384-wide slice of D_FF for ALL 8 experts (same 9.4MB of fp16 weights per core
as expert-parallel) and runs the FFN for ALL routed token-expert pairs over
its slice, one expert "phase" at a time. Each core therefore does exactly
sum(C_e)*36 matmul-cycles -- perfect load balance with zero padding (vs
8*max(C_e)*36 for expert-parallel). The host sums the 8 partial outputs
(fp16) and applies the softmax-weighted combine + b2.

Device layout puts tokens on the matmul free axis, so both matmuls contract
over the partition axis with zero on-device transposes:
    HT[f,t] = relu(sum_d W1[d,f] * XT[d,t] + b1[f])   lhsT=W1, rhs=XT
    YT[d,t] =      sum_{f in slice} W2[f,d] * HT[f,t] lhsT=W2, rhs=HT
The chunk loop is software-pipelined (MM1 of chunk i+1 is emitted before
MM2 of chunk i) so the relu latency never stalls the PE. MM2's PSUM
accumulation is only 3 deep here, so the drain load is 8x expert-parallel's;
relu runs on ACT and the MM2 drains split 4:2 between DVE and ACT (Pool
cannot read PSUM). Tokens stream on the sync DMA queue, weights on the DVE
queue (parallel transfer in the prologue), outputs issue from Pool. The
last items' outputs go out per-2ko so the final transfers overlap the tail.
"""

import numpy as np

import concourse.tile as tile
from concourse import bacc, mybir
from concourse import bass_utils

D_MODEL = 768
N_EXPERTS = 8
TOP_K = 2
D_FF = 3072
P = 128
KO = D_MODEL // P     # 6   contraction chunks for MM1 / output tiles for MM2
FS = 3                # f-tiles per core slice (384 of 3072 FF columns)
FSP = FS * P          # 384
N0 = 256              # first-phase chunk0 (small so its tokens land early)
WARMUP_MMS = 38       # small 128-row dummy matmuls bridge the DMA prologue
TAIL_SPLIT = 3        # last N work items DMA their output per-2ko

_program_cache: dict[tuple, object] = {}


def _phase_chunks(C, first):
    """Chunks (t0, nt) covering C tokens, each <=512. The first phase opens
    with a small N0 chunk so the critical DMAs land early."""
    chunks = []
    t = 0
    if first and C > N0:
        chunks.append((0, N0))
        t = N0
    while t < C:
        n = min(512, C - t)
        chunks.append((t, n))
        t += n
    return chunks


def _build_program(counts):
    """Bass program: all 8 expert phases over this core's FF slice (SPMD x8).

    counts: per-phase token counts (already padded to x4)."""
    key = tuple(counts)
    if key in _program_cache:
        return _program_cache[key]

    fp16 = mybir.dt.float16
    fp32 = mybir.dt.float32
    nc = bacc.Bacc("TRN2", target_bir_lowering=False, debug=False,
                   enable_asserts=True, num_devices=N_EXPERTS)

    cmax = max(counts)
    c0 = counts[0]
    # wb_e layout: w1_e (ko-major [KO, FSP]) then w2_e ([FS, D])
    off_w2 = KO * FSP
    lw = off_w2 + FS * D_MODEL

    # phase-0 prologue bundles: per-DMA latency is ~1.5us, so the critical
    # path wants FEW, FAT transfers. wb0a = w1_0 + all biases (fp16); wb0b =
    # w2_0. Tokens ride the other (scalar) queue in parallel as xta|xtb.
    off_b1 = KO * FSP
    l0a = off_b1 + N_EXPERTS * FS
    xta_d = nc.dram_tensor("xta", [P, KO, N0], fp16,
                           kind="ExternalInput").ap()
    xtb_d = nc.dram_tensor("xtb", [P, KO, c0 - N0], fp16,
                           kind="ExternalInput").ap()
    wb0a_d = nc.dram_tensor("wb0a", [P, l0a], fp16,
                            kind="ExternalInput").ap()
    wb0b_d = nc.dram_tensor("wb0b", [P, FS, D_MODEL], fp16,
                            kind="ExternalInput").ap()
    wb_d = [None] + [
        nc.dram_tensor(f"wb{e}", [P, lw], fp16, kind="ExternalInput").ap()
        for e in range(1, N_EXPERTS)]
    xt_d = [None] + [
        nc.dram_tensor(f"xt{e}", [P, KO, counts[e]], fp16,
                       kind="ExternalInput").ap()
        for e in range(1, N_EXPERTS)]
    yt_d = [nc.dram_tensor(f"yt{e}", [P, KO, counts[e]], fp16,
                           kind="ExternalOutput").ap()
            for e in range(N_EXPERTS)]

    with tile.TileContext(nc) as tc:
        with (
            tc.tile_pool(name="wpool", bufs=1) as wpool,
            tc.tile_pool(name="xtpool", bufs=4) as xtpool,
            tc.tile_pool(name="hpool", bufs=2) as hpool,
            tc.tile_pool(name="ypool", bufs=3) as ypool,
            tc.tile_pool(name="pspool", bufs=7, space="PSUM") as pspool,
        ):
            xta_sb = wpool.tile([P, KO, N0], fp16)
            xtb_sb = wpool.tile([P, KO, c0 - N0], fp16)
            wb0a_sb = wpool.tile([P, l0a], fp16)
            wb0b_sb = wpool.tile([P, FS, D_MODEL], fp16)
            wb_sb = [None] + [wpool.tile([P, lw], fp16, name=f"wb_sb{e}")
                              for e in range(1, N_EXPERTS)]

            def w1_slice(e, ft, ko):
                """AP of w1 f-tile `ft` (local), contraction chunk ko."""
                o = ko * FSP + ft * P
                if e == 0:
                    return wb0a_sb[:, o:o + P]
                return wb_sb[e][:, o:o + P]

            def w2_slice(e, ft, ko):
                if e == 0:
                    return wb0b_sb[:, ft, ko * P:(ko + 1) * P]
                o = off_w2 + ft * D_MODEL + ko * P
                return wb_sb[e][:, o:o + P]

            def b1_ap(e, ft):
                o = off_b1 + e * FS + ft
                return wb0a_sb[:, o:o + 1]

            # PE warmup: small dummy matmuls bridge the DMA prologue and keep
            # the HAM clock ramping; small so the first real matmul slots in
            # quickly once the critical DMAs land.
            warm = wpool.tile([P, P], fp16)
            hdummy = wpool.tile([P, 8], fp16)
            nc.gpsimd.memset(warm[:], 0.0)
            # dummy activation: forces the 1.3us ACT_TABLE_LOAD into the
            # prologue instead of blocking the first real relu
            nc.scalar.activation(hdummy[:], warm[:, :8],
                                 mybir.ActivationFunctionType.Relu)
            ps_w = pspool.tile([P, 512], fp32, name="ps_w", bufs=1)
            for _ in range(WARMUP_MMS):
                nc.tensor.matmul(ps_w[:, :P], lhsT=warm[:], rhs=warm[:],
                                 start=True, stop=True)

            # Input DMAs: tokens on the sync queue, weights on the scalar
            # (ACT) queue, each in stream (need) order; the two queues
            # transfer in parallel so the first phase's tokens AND weights
            # land early. Only phase-0's small weight pieces are issued
            # up-front; w2a and the wb bundles are issued from inside the
            # compute stream (one phase ahead) so they don't delay the
            # first relus on the ACT queue. The xt pool's bufs=4 rotation
            # paces the token stream.
            nc.scalar.dma_start(xta_sb[:], xta_d[:])
            nc.scalar.dma_start(xtb_sb[:], xtb_d[:])
            nc.sync.dma_start(wb0a_sb[:], wb0a_d[:])
            nc.sync.dma_start(wb0b_sb[:], wb0b_d[:])
            # xt tiles are allocated here (pool rotation order) but their
            # DMAs are issued from inside the compute stream, a phase ahead,
            # so they don't steal prologue bandwidth from the critical path
            xt_sb = [None] * N_EXPERTS
            for e in range(1, N_EXPERTS):
                xt_sb[e] = xtpool.tile([P, KO, cmax], fp16, name="xt")

            def xt_rhs(e, ko, t0, nt):
                if e == 0:
                    if t0 < N0:
                        return xta_sb[:, ko, t0:t0 + nt]
                    return xtb_sb[:, ko, t0 - N0:t0 - N0 + nt]
                return xt_sb[e][:, ko, t0:t0 + nt]

            # work items: (phase, t0, nt), software-pipelined one deep
            items = []
            first_item = {}
            fidx = {}
            for e in range(N_EXPERTS):
                for (t0, nt) in _phase_chunks(counts[e], first=(e == 0)):
                    if t0 == 0:
                        first_item[len(items)] = e
                        fidx[e] = len(items)
                    items.append((e, t0, nt))
            # anchor item at which each phase's token DMA is issued (a phase
            # ahead of need; phase 1's waits one extra chunk so it doesn't
            # steal prologue bandwidth from the critical w1/xt transfers)
            issue_xt = {}
            for p in range(1, N_EXPERTS):
                anchor = fidx[p - 1] + 1 if p == 1 else fidx[p - 1]
                issue_xt.setdefault(anchor, []).append(p)

            hts = {}

            def emit_mm1(i):
                e, t0, nt = items[i]
                ht = hpool.tile([P, FS, 512], fp16, name="ht")
                hts[i] = ht
                for ft in range(FS):
                    ps = pspool.tile([P, 512], fp32, name="ps")
                    for ko in range(KO):
                        nc.tensor.matmul(
                            ps[:, :nt],
                            lhsT=w1_slice(e, ft, ko),
                            rhs=xt_rhs(e, ko, t0, nt),
                            start=(ko == 0), stop=(ko == KO - 1),
                        )
                    nc.scalar.activation(
                        ht[:, ft, :nt], ps[:, :nt],
                        mybir.ActivationFunctionType.Relu,
                        bias=b1_ap(e, ft),
                    )
                # issue the next phase's weight bundle from the ACT queue
                # and its tokens from the sync queue (behind this item's
                # relus, one phase ahead of need)
                if i in first_item:
                    p = first_item[i]
                    if p + 1 < N_EXPERTS:
                        nc.scalar.dma_start(wb_sb[p + 1][:], wb_d[p + 1][:])
                for p in issue_xt.get(i, ()):
                    nc.sync.dma_start(xt_sb[p][:, :, :counts[p]], xt_d[p][:])

            def emit_mm2(i):
                e, t0, nt = items[i]
                ht = hts.pop(i)
                tail = i >= len(items) - TAIL_SPLIT
                yt = ypool.tile([P, KO, 512], fp16, name="yt")
                for ko in range(KO):
                    ps = pspool.tile([P, 512], fp32, name="ps")
                    for ft in range(FS):
                        nc.tensor.matmul(
                            ps[:, :nt],
                            lhsT=w2_slice(e, ft, ko),
                            rhs=ht[:, ft, :nt],
                            start=(ft == 0), stop=(ft == FS - 1),
                        )
                    # MM2's drain load is heavy (3-deep accumulation) and
                    # Pool can't read PSUM: split it 4:2 across DVE and ACT
                    if ko % 3:
                        nc.vector.tensor_copy(yt[:, ko, :nt], ps[:, :nt])
                    else:
                        nc.scalar.activation(
                            yt[:, ko, :nt], ps[:, :nt],
                            mybir.ActivationFunctionType.Copy)
                    if tail and ko % 2 == 1:
                        # tail items stream their output out per-2ko, spread
                        # over three otherwise-idle queues, so the final
                        # transfers and their issue overlap remaining compute
                        eng = (nc.sync, nc.gpsimd, nc.scalar)[ko // 2]
                        eng.dma_start(
                            yt_d[e][:, ko - 1:ko + 1, t0:t0 + nt],
                            yt[:, ko - 1:ko + 1, :nt])
                if not tail:
                    nc.gpsimd.dma_start(yt_d[e][:, :, t0:t0 + nt],
                                        yt[:, :, :nt])

            emit_mm1(0)
            for i in range(len(items) - 1):
                emit_mm1(i + 1)
                emit_mm2(i)
            emit_mm2(len(items) - 1)

    nc.compile()
    _program_cache[key] = nc
    return nc


def _route(xf, Wr):
    """Host router: top-2 expert ids + softmax weights (matches lax.top_k)."""
    T = xf.shape[0]
    logits = xf @ Wr
    i1 = np.argmax(logits, axis=1)
    l1 = logits[np.arange(T), i1]
    masked = logits.copy()
    masked[np.arange(T), i1] = -np.inf
    i2 = np.argmax(masked, axis=1)
    l2 = logits[np.arange(T), i2]
    e2 = np.exp((l2 - l1).astype(np.float32))
    wt1 = 1.0 / (1.0 + e2)
    wt2 = e2 / (1.0 + e2)
    return i1, i2, wt1, wt2


def _forward(inputs, trace=False, trace_kwargs=None):
    x = np.ascontiguousarray(np.asarray(inputs["x"], dtype=np.float32))
    Wr = np.asarray(inputs["Wr"], dtype=np.float32)
    W1 = np.asarray(inputs["W1"], dtype=np.float32)
    b1 = np.asarray(inputs["b1"], dtype=np.float32)
    W2 = np.asarray(inputs["W2"], dtype=np.float32)
    b2 = np.asarray(inputs["b2"], dtype=np.float32)

    B, S, D = x.shape
    T = B * S
    xf = x.reshape(T, D)

    i1, i2, wt1, wt2 = _route(xf, Wr)
    idx = [np.nonzero((i1 == e) | (i2 == e))[0] for e in range(N_EXPERTS)]
    gw = [np.where(i1[ix] == e, wt1[ix], wt2[ix]).astype(np.float32)
          for e, ix in enumerate(idx)]

    # phase order: the last phase ends the kernel, so give it the smallest
    # trailing chunk (fast drain tail)
    counts = [max(-(-len(ix) // 4) * 4, 4) for ix in idx]

    def rem(c):
        r = c % 512
        return r if r else 512
    order = list(range(N_EXPERTS))
    last = min(order, key=lambda e: rem(counts[e]))
    order.remove(last)
    order.append(last)

    pcounts = [counts[e] for e in order]
    nc = _build_program(tuple(pcounts))

    # per-phase token tensors (identical for every core)
    xts = []
    for p, e in enumerate(order):
        ix = idx[e]
        C = pcounts[p]
        xe = np.zeros((C, D), dtype=np.float16)
        xe[:len(ix)] = xf[ix]
        # XT[d,t] -> [p, ko, t] with d = ko*P + p
        xts.append(np.ascontiguousarray(
            xe.T.reshape(KO, P, C).transpose(1, 0, 2)))

    in_maps = []
    for c in range(N_EXPERTS):
        fsl = slice(c * FSP, (c + 1) * FSP)
        m = {}
        w1s, w2s, b1s = [], [], []
        for p, e in enumerate(order):
            w1s.append(np.ascontiguousarray(
                W1[e].astype(np.float16).reshape(KO, P, D_FF)[:, :, fsl]
                .transpose(1, 0, 2)))       # [P, KO, FSP]
            w2s.append(np.ascontiguousarray(
                W2[e].astype(np.float16)
                .reshape(D_FF // P, P, D_MODEL)[c * FS:(c + 1) * FS]
                .transpose(1, 0, 2)))       # [P, FS, D]
            b1s.append(b1[e][fsl].reshape(FS, P).T)
        m["xta"] = np.ascontiguousarray(xts[0][:, :, :N0])
        m["xtb"] = np.ascontiguousarray(xts[0][:, :, N0:])
        m["wb0a"] = np.ascontiguousarray(np.concatenate(
            [w1s[0].reshape(P, -1),
             np.concatenate(b1s, axis=1).astype(np.float16)], axis=1))
        m["wb0b"] = w2s[0]
        for p in range(1, N_EXPERTS):
            m[f"wb{p}"] = np.ascontiguousarray(np.concatenate(
                [w1s[p].reshape(P, -1), w2s[p].reshape(P, -1)], axis=1))
            m[f"xt{p}"] = xts[p]
        in_maps.append(m)

    try:
        res = bass_utils.run_bass_kernel_spmd(
            nc, in_maps, core_ids=list(range(N_EXPERTS)), trace=trace,
            **(trace_kwargs or {}),
        )
    except Exception:
        # transient device errors (NRT_EXEC_UNIT_UNRECOVERABLE) have been
        # observed once under rapid successive loads; one retry clears them
        res = bass_utils.run_bass_kernel_spmd(
            nc, in_maps, core_ids=list(range(N_EXPERTS)), trace=trace,
            **(trace_kwargs or {}),
        )

    out = np.zeros((T, D), dtype=np.float32)
    for p, e in enumerate(order):
        ix = idx[e]
        if len(ix) == 0:
            continue
        # sum the 8 cores' fp16 partials: yt [p, ko, t] -> Y [t, d]
        yt = res.results[0][f"yt{p}"].astype(np.float32)
        for c in range(1, N_EXPERTS):
            yt += res.results[c][f"yt{p}"].astype(np.float32)
        ye = yt.transpose(2, 1, 0).reshape(pcounts[p], D)[:len(ix)]
        out[ix] += gw[e][:, None] * (ye + b2[e][None, :])
    return out.reshape(B, S, D), res


def kernel(**inputs) -> np.ndarray:
    out, _ = _forward(inputs)
    return out
